# revision 1
# baseline (speedup 1.0000x reference)
"""Bass/Tile Trainium2 kernel for nn_Attention (B=4, T=4096, C=256), 8 cores.

Sharding: core = (batch b, query-half h). Each core computes the full K/V
projections for its batch and attention output for its 2048 query rows.

Layout strategy (all matmuls bf16, fp32 PSUM accumulation):
  - Host pre-transposes x to x^T [C, T]; projections contract C on
    partitions. k^T/q^T come out feature-major, so the score matmul
    produces scoresT [keys j on partitions, queries q on free dim].
  - Softmax needs no max-subtraction (scores are O(1); exp cannot
    overflow fp32) and no partition reductions.
  - The 0/1 key mask is folded in on the host by zeroing masked key
    columns of x^T: k and v rows of masked keys become 0, and the
    appended ones column of V is masked on-device, so masked keys drop
    out of both softmax sums and exp needs no bias at all. The torch
    quirk (+1.0 bias on valid keys) cancels in softmax.
  - V gets a column of ones appended: out[q, 256] accumulates the
    softmax denominator for free. Final: out[:, :256] * (1/out[:, 256]).
  - Main loop is software-pipelined per key block: PE does the two score
    matmuls for block jb+1 and then the four out-matmuls for block jb,
    so ACT's exp (720 ns/tile) hides behind ~1.2 us of PE work.
"""

import numpy as np
import ml_dtypes

import concourse.bacc as bacc
import concourse.mybir as mybir
import concourse.tile as tile
from concourse.bass_utils import run_bass_kernel_spmd

B, T, C = 4, 4096, 256
NCORES = 8
HALVES = NCORES // B          # 2 query-halves per batch
TQ = T // HALVES              # 2048 query rows per core
PB = 128                      # partition block
NCCH = C // PB                # 2 contraction chunks of 128
NJB = T // PB                 # 32 key blocks
SBW = 512                     # query superblock width
NSB = TQ // SBW               # 4 superblocks per core
NQB = SBW // PB               # 4 query 128-blocks per superblock
VW = C + 1                    # v tile width incl. ones column
SCALE = float(C) ** -0.5
BF16 = mybir.dt.bfloat16
F32 = mybir.dt.float32
FP8 = mybir.dt.float8e4
VWP = 272                     # fp8 va block pitch (16B-aligned for DoubleRow)
FP8_EXP_BIAS = -6.0           # exp shift so p fits fp8e4m3 range; cancels in softmax


def _emit(tc, out, xt, xq, wq, wk, wv, mb, mode="full", fp8=False):
    nc = tc.nc
    import contextlib

    with contextlib.ExitStack() as ctx:
        persist = ctx.enter_context(tc.tile_pool(name="persist", bufs=1))
        # Persistent SBUF tensors; c-chunks laid side by side on the free dim.
        xt_sb = persist.tile([PB, NCCH * T], BF16)    # x^T  (full batch seq)
        xq_sb = persist.tile([PB, NCCH * TQ], BF16)   # x^T  (this core's half)
        wq_sb = persist.tile([PB, NCCH * C], BF16)
        wk_sb = persist.tile([PB, NCCH * C], BF16)
        wv_sb = persist.tile([PB, NCCH * C], BF16)
        kt_sb = persist.tile([PB, NCCH * T], BF16)    # k^T
        qt_sb = persist.tile([PB, NCCH * TQ], BF16)   # q^T
        vdt, vw = (FP8, VWP) if fp8 else (BF16, VW)
        va_sb = persist.tile([PB, NJB * vw], vdt)     # masked v + masked ones col
        mb_sb = persist.tile([PB, NJB], F32)          # 0/1 mask, [j in block, jb]

        # Few, large, descriptor-friendly DMAs spread across the three
        # DMA-capable queues (sync/scalar HWDGE, gpsimd SWDGE). xq and
        # weights land first so the q projection starts while xt streams.
        w2 = lambda w: w.rearrange("(n p) c -> p n c", p=PB)
        s3 = lambda t, n: t.rearrange("p (n c) -> p n c", n=n)
        dma_v2 = globals().get("DMA_V2", True)
        if dma_v2:
            nc.scalar.dma_start(s3(wq_sb[:], NCCH), w2(wq))
            nc.scalar.dma_start(s3(wk_sb[:], NCCH), w2(wk))
            nc.gpsimd.dma_start(s3(wv_sb[:], NCCH), w2(wv))
            nc.gpsimd.dma_start(mb_sb[:], mb)
            nc.sync.dma_start(s3(xq_sb[:], NCCH),
                              xq.rearrange("(n p) t -> p n t", p=PB))
            H = T // 2
            nc.sync.dma_start(xt_sb[:, 0:H], xt[0:PB, 0:H])
            nc.scalar.dma_start(xt_sb[:, T:T + H], xt[PB:2 * PB, 0:H])
            nc.sync.dma_start(xt_sb[:, H:T], xt[0:PB, H:T])
            nc.scalar.dma_start(xt_sb[:, T + H:2 * T], xt[PB:2 * PB, H:T])
        else:
            nc.scalar.dma_start(s3(wq_sb[:], NCCH), w2(wq))
            nc.sync.dma_start(s3(wk_sb[:], NCCH), w2(wk))
            nc.gpsimd.dma_start(s3(wv_sb[:], NCCH), w2(wv))
            nc.sync.dma_start(mb_sb[:], mb)
            nc.gpsimd.dma_start(s3(xq_sb[:], NCCH),
                                xq.rearrange("(n p) t -> p n t", p=PB))
            nc.sync.dma_start(xt_sb[:, 0:T], xt[0:PB, :])
            nc.scalar.dma_start(xt_sb[:, T:2 * T], xt[PB:2 * PB, :])

        if fp8:
            fp8_bias = persist.tile([PB, 1], F32, name="fp8_bias")
            nc.vector.memset(fp8_bias[:], FP8_EXP_BIAS)
        # masked ones column: va[:, jb*vw + C] = mask01[:, jb]
        va_ones = va_sb[:].rearrange("p (j e) -> p j e", e=vw)[:, :, C:C + 1]
        nc.vector.tensor_copy(va_ones, mb_sb[:].rearrange("p (j e) -> p j e", e=1))

        # ---- projections ----
        with tc.tile_pool(name="proj_psum", bufs=2, space="PSUM") as pp:
            # q^T[d, t] / k^T[d, t]: lhsT = W^T chunk [c, d], rhs = x^T [c, t]
            for w_sb, x_src, x_w, dst, copy_eng in (
                (wq_sb, xq_sb, TQ, qt_sb, nc.vector.tensor_copy),
                (wk_sb, xt_sb, T, kt_sb, nc.scalar.copy),
            ):
                for s in range(x_w // 512):
                    for dc in range(NCCH):
                        ps = pp.tile([PB, 512], F32, tag="proj", name="proj_ps")
                        for cc in range(NCCH):
                            nc.tensor.matmul(
                                ps,
                                lhsT=w_sb[:, cc * C + dc * PB: cc * C + (dc + 1) * PB],
                                rhs=x_src[:, cc * x_w + s * 512: cc * x_w + (s + 1) * 512],
                                start=(cc == 0),
                                stop=(cc == NCCH - 1),
                            )
                        copy_eng(dst[:, dc * x_w + s * 512: dc * x_w + (s + 1) * 512], ps)
            # v[t, d]: lhsT = x^T chunk [c, t-block], rhs = W^T chunk [c, d].
            # xt is host-masked (masked key columns zeroed), so v rows and
            # the ones column carry the mask; no device-side masking here.
            for jb in range(NJB):
                ps = pp.tile([PB, C], F32, tag="projv", name="projv_ps")
                for cc in range(NCCH):
                    nc.tensor.matmul(
                        ps,
                        lhsT=xt_sb[:, cc * T + jb * PB: cc * T + (jb + 1) * PB],
                        rhs=wv_sb[:, cc * C:(cc + 1) * C],
                        start=(cc == 0),
                        stop=(cc == NCCH - 1),
                    )
                nc.vector.tensor_copy(va_sb[:, jb * vw: jb * vw + C], ps)

        # ---- attention main loop ----
        scp = ctx.enter_context(tc.tile_pool(name="sc_psum", bufs=3, space="PSUM"))
        op = ctx.enter_context(tc.tile_pool(name="o_psum", bufs=1, space="PSUM"))
        ppool = ctx.enter_context(tc.tile_pool(name="p_pool", bufs=4))
        fin = ctx.enter_context(tc.tile_pool(name="fin", bufs=3))

        if mode == "projonly":
            os_t = fin.tile([PB, C], F32, tag="os", name="os_t")
            nc.vector.tensor_copy(os_t, kt_sb[:, 0:C])
            nc.sync.dma_start(out[0:PB, :], os_t)
            return
        if mode == "noscores":
            p_static = persist.tile([PB, 4 * SBW], BF16, name="p_static")
            nc.vector.memset(p_static[:], 1.0)

        for sb in range(NSB):
            if mode == "noout":
                op_tiles = None
            else:
                op_tiles = [op.tile([PB, VW], F32, tag=f"o{qb}", name=f"opsum{qb}",
                                    bufs=2 if qb == 0 else 1)
                            for qb in range(NQB)]
            p_tiles = {}

            def emit_scores(jb, sb=sb, p_tiles=p_tiles):
                ps = scp.tile([PB, SBW], F32, tag="sc", name="sc_ps")
                for cc in range(NCCH):
                    nc.tensor.matmul(
                        ps,
                        lhsT=kt_sb[:, cc * T + jb * PB: cc * T + (jb + 1) * PB],
                        rhs=qt_sb[:, cc * TQ + sb * SBW: cc * TQ + (sb + 1) * SBW],
                        start=(cc == 0),
                        stop=(cc == NCCH - 1),
                    )
                if fp8:
                    # p for a key-block pair lives in one [128, 2*SBW] tile so
                    # the pair forms a DoubleRow stationary [128, 2, 128].
                    if jb % 2 == 0:
                        pt = ppool.tile([PB, 2 * SBW], FP8, tag="p", name="p_t")
                        p_tiles[jb // 2] = pt
                    else:
                        pt = p_tiles[jb // 2]
                    nc.scalar.activation(
                        pt[:, (jb % 2) * SBW:(jb % 2 + 1) * SBW], ps,
                        mybir.ActivationFunctionType.Exp,
                        bias=fp8_bias[:], scale=SCALE)
                else:
                    pt = ppool.tile([PB, SBW], BF16, tag="p", name="p_t")
                    nc.scalar.activation(
                        pt, ps, mybir.ActivationFunctionType.Exp, scale=SCALE)
                    p_tiles[jb] = pt

            def emit_out(jb, op_tiles=op_tiles, p_tiles=p_tiles):
                pt = p_tiles.pop(jb)
                for qb in range(NQB):
                    nc.tensor.matmul(
                        op_tiles[qb],
                        lhsT=pt[:, qb * PB:(qb + 1) * PB],
                        rhs=va_sb[:, jb * VW:(jb + 1) * VW],
                        start=(jb == 0),
                        stop=(jb == NJB - 1),
                    )

            def emit_out_fp8(jp, op_tiles=op_tiles, p_tiles=p_tiles):
                # one DoubleRow matmul contracts both key blocks of the pair
                pt = p_tiles.pop(jp)
                pt3 = pt[:].rearrange("p (n c) -> p n c", n=2)
                va3 = va_sb[:, 2 * jp * VWP:(2 * jp + 2) * VWP].rearrange(
                    "p (n c) -> p n c", n=2)
                for qb in range(NQB):
                    nc.tensor.matmul(
                        op_tiles[qb],
                        lhsT=pt3[:, :, qb * PB:(qb + 1) * PB],
                        rhs=va3[:, :, 0:VW],
                        start=(jp == 0),
                        stop=(jp == NJB // 2 - 1),
                        perf_mode=mybir.MatmulPerfMode.DoubleRow,
                    )

            if mode == "noout":
                for jb in range(NJB):
                    emit_scores(jb)
                    p_tiles.pop(jb)
            elif mode == "noscores":
                for jb in range(NJB):
                    for qb in range(NQB):
                        nc.tensor.matmul(
                            op_tiles[qb],
                            lhsT=p_static[:, (jb % 4) * SBW + qb * PB:
                                          (jb % 4) * SBW + (qb + 1) * PB],
                            rhs=va_sb[:, jb * VW:(jb + 1) * VW],
                            start=(jb == 0),
                            stop=(jb == NJB - 1),
                        )
            else:
                # software-pipelined: scores/exp for jp+1 are emitted before
                # the out-matmuls of jp so PE never stalls on ACT.
                if fp8:
                    emit_scores(0)
                    emit_scores(1)
                    for jp in range(NJB // 2):
                        if 2 * jp + 2 < NJB:
                            emit_scores(2 * jp + 2)
                            emit_scores(2 * jp + 3)
                        emit_out_fp8(jp)
                else:
                    emit_scores(0)
                    for jb in range(NJB):
                        if jb + 1 < NJB:
                            emit_scores(jb + 1)
                        emit_out(jb)
            if mode == "noout":
                os_t = fin.tile([PB, C], F32, tag="os", name="os_t")
                nc.vector.tensor_copy(os_t, kt_sb[:, sb * C:(sb + 1) * C])
                nc.sync.dma_start(out[sb * PB:(sb + 1) * PB, :], os_t)
                continue
            os_t = fin.tile([PB, NQB * C], F32, tag="os", name="os_t")
            for qb in range(NQB):
                rec = fin.tile([PB, 1], F32, tag="rec", name="rec_t")
                nc.vector.reciprocal(rec, op_tiles[qb][:, C:C + 1])
                nc.vector.tensor_scalar_mul(
                    os_t[:, qb * C:(qb + 1) * C], op_tiles[qb][:, 0:C], rec)
            dma_eng = nc.sync if sb % 2 == 0 else nc.scalar
            dma_eng.dma_start(
                out[sb * SBW:(sb + 1) * SBW, :].rearrange("(q p) c -> p q c", p=PB),
                os_t[:].rearrange("p (q c) -> p q c", q=NQB))


def build_nc(reps=1, loop_n=0, mode="full", fp8=False):
    nc = bacc.Bacc("TRN2", target_bir_lowering=False, debug=False)
    xt = nc.dram_tensor("xt", [C, T], BF16, kind="ExternalInput").ap()
    xq = nc.dram_tensor("xq", [C, TQ], BF16, kind="ExternalInput").ap()
    wq = nc.dram_tensor("wq", [C, C], BF16, kind="ExternalInput").ap()
    wk = nc.dram_tensor("wk", [C, C], BF16, kind="ExternalInput").ap()
    wv = nc.dram_tensor("wv", [C, C], BF16, kind="ExternalInput").ap()
    mb = nc.dram_tensor("mb", [PB, NJB], F32, kind="ExternalInput").ap()
    out = nc.dram_tensor("out", [TQ, C], F32, kind="ExternalOutput").ap()
    with tile.TileContext(nc) as tc:
        if loop_n:
            with tc.For_i(0, loop_n, 1, hint_engines=(mybir.EngineType.PE,)):
                _emit(tc, out, xt, xq, wq, wk, wv, mb, mode=mode, fp8=fp8)
        else:
            for _ in range(reps):
                _emit(tc, out, xt, xq, wq, wk, wv, mb, mode=mode, fp8=fp8)
    nc.compile()
    return nc


_CACHE = {}


def _get_nc():
    if "nc" not in _CACHE:
        _CACHE["nc"] = build_nc()
    return _CACHE["nc"]


def make_in_maps(x, mask):
    bf = ml_dtypes.bfloat16
    x = np.asarray(x, dtype=np.float32)
    xt_all = np.ascontiguousarray(x.transpose(0, 2, 1)).astype(bf)  # [B, C, T]
    m01 = (np.asarray(mask) != 0).astype(np.float32)                # [B, T]
    # zero the masked key columns of x^T: k/v of masked keys become 0, and
    # with the masked ones column they drop out of both softmax sums.
    xtm_all = (xt_all.astype(np.float32) * m01[:, None, :]).astype(bf)
    maps = []
    for core in range(NCORES):
        b, h = divmod(core, HALVES)
        maps.append({
            "xt": xtm_all[b],
            "xq": np.ascontiguousarray(xt_all[b][:, h * TQ:(h + 1) * TQ]),
            "mb": np.ascontiguousarray(m01[b].reshape(NJB, PB).T),
        })
    return maps


def kernel(x, mask, Wk, Wq, Wv):
    bf = ml_dtypes.bfloat16
    wqt = np.ascontiguousarray(np.asarray(Wq, dtype=np.float32).T).astype(bf)
    wkt = np.ascontiguousarray(np.asarray(Wk, dtype=np.float32).T).astype(bf)
    wvt = np.ascontiguousarray(np.asarray(Wv, dtype=np.float32).T).astype(bf)
    in_maps = make_in_maps(x, mask)
    for m in in_maps:
        m.update({"wq": wqt, "wk": wkt, "wv": wvt})
    res = run_bass_kernel_spmd(_get_nc(), in_maps, list(range(NCORES)))
    out = np.empty((B, T, C), np.float32)
    for core in range(NCORES):
        b, h = divmod(core, HALVES)
        out[b, h * TQ:(h + 1) * TQ, :] = res.results[core]["out"]
    return out



# revision 3
# speedup vs baseline: 1.3596x; 1.3596x over previous
"""Bass/Tile Trainium2 kernel for nn_Attention (B=4, T=4096, C=256), 8 cores.

Sharding: core = (batch b, query-half h). Each core computes the full K/V
projections for its batch and attention output for its 2048 query rows.

Key compaction: the 0/1 key mask keeps ~50% of keys. The host gathers the
valid key columns of x^T per batch (padded with zeros to TK), so the device
only projects/attends over TK=2176 keys instead of T=4096 — softmax over
the compacted key set is exact (the torch +1.0-on-valid-keys quirk is a
uniform shift that cancels; padding keys have v=0 and a zeroed ones-column
entry so they drop out of both softmax sums). Falls back to a full-T build
if a batch ever has more than TK valid keys.

Layout strategy (all matmuls bf16, fp32 PSUM accumulation):
  - Host pre-transposes x to x^T [C, T]; projections contract C on
    partitions. k^T/q^T come out feature-major, so the score matmul
    produces scoresT [keys j on partitions, queries q on free dim].
  - Softmax needs no max-subtraction (scores are O(1); exp cannot
    overflow fp32) and no partition reductions.
  - V gets a column of ones appended: out[q, 256] accumulates the
    softmax denominator for free. Final: out[:, :256] * (1/out[:, 256]).
  - Main loop is software-pipelined per key block: PE does the two score
    matmuls for block jb+1 and then the four out-matmuls for block jb,
    so ACT's exp (~720 ns/tile) hides behind PE work.
"""

import numpy as np
import ml_dtypes

import concourse.bacc as bacc
import concourse.mybir as mybir
import concourse.tile as tile
from concourse.bass_utils import run_bass_kernel_spmd

B, T, C = 4, 4096, 256
NCORES = 8
HALVES = NCORES // B          # 2 query-halves per batch
TQ = T // HALVES              # 2048 query rows per core
PB = 128                      # partition block
NCCH = C // PB                # 2 contraction chunks of 128
TK = 2176                     # compacted+padded key count (17 blocks of 128)
SBW = 512                     # query superblock width
NSB = TQ // SBW               # 4 superblocks per core
NQB = SBW // PB               # 4 query 128-blocks per superblock
VW = C + 1                    # v tile width incl. ones column
SCALE = float(C) ** -0.5
BF16 = mybir.dt.bfloat16
F32 = mybir.dt.float32
FP8 = mybir.dt.float8e4
VWP = 272                     # fp8 va block pitch (16B-aligned for DoubleRow)
FP8_EXP_BIAS = -6.0           # exp shift so p fits fp8e4m3 range; cancels in softmax


def _emit(tc, out, xt, xq, wq, wk, wv, mb, tk, mode="full", fp8=False):
    nc = tc.nc
    import contextlib
    njb = tk // PB            # key blocks
    nks = tk // 512           # full 512-wide k-proj blocks
    ktail = tk - nks * 512    # k-proj tail width (multiple of 128)

    with contextlib.ExitStack() as ctx:
        persist = ctx.enter_context(tc.tile_pool(name="persist", bufs=1))
        # Persistent SBUF tensors; c-chunks laid side by side on the free dim.
        xt_sb = persist.tile([PB, NCCH * tk], BF16)   # x^T  (compacted keys)
        xq_sb = persist.tile([PB, NCCH * TQ], BF16)   # x^T  (this core's half)
        wq_sb = persist.tile([PB, NCCH * C], BF16)
        wk_sb = persist.tile([PB, NCCH * C], BF16)
        wv_sb = persist.tile([PB, NCCH * C], BF16)
        kt_sb = persist.tile([PB, NCCH * tk], BF16)   # k^T
        qt_sb = persist.tile([PB, NCCH * TQ], BF16)   # q^T
        vdt, vw = (FP8, VWP) if fp8 else (BF16, VW)
        va_sb = persist.tile([PB, njb * vw], vdt)     # masked v + masked ones col
        mb_sb = persist.tile([PB, njb], F32)          # 0/1 mask, [j in block, jb]

        # Few, large, descriptor-friendly DMAs spread across the three
        # DMA-capable queues (sync/scalar HWDGE, gpsimd SWDGE). xq and
        # weights land first so the q projection starts while xt streams.
        w2 = lambda w: w.rearrange("(n p) c -> p n c", p=PB)
        s3 = lambda t, n: t.rearrange("p (n c) -> p n c", n=n)
        nc.scalar.dma_start(s3(wq_sb[:], NCCH), w2(wq))
        nc.scalar.dma_start(s3(wk_sb[:], NCCH), w2(wk))
        nc.gpsimd.dma_start(s3(wv_sb[:], NCCH), w2(wv))
        nc.gpsimd.dma_start(mb_sb[:], mb)
        HQ = TQ // 2
        xq3 = xq.rearrange("(n p) t -> p n t", p=PB)
        # xq in two halves so the q-projection's first blocks start early
        for hh in range(2):
            dst = xq_sb[:].rearrange("p (n t) -> p n t", n=NCCH)[:, :, hh * HQ:(hh + 1) * HQ]
            nc.sync.dma_start(dst, xq3[:, :, hh * HQ:(hh + 1) * HQ])
        H = tk // 2
        nc.sync.dma_start(xt_sb[:, 0:H], xt[0:PB, 0:H])
        nc.scalar.dma_start(xt_sb[:, tk:tk + H], xt[PB:2 * PB, 0:H])
        nc.sync.dma_start(xt_sb[:, H:tk], xt[0:PB, H:tk])
        nc.scalar.dma_start(xt_sb[:, tk + H:2 * tk], xt[PB:2 * PB, H:tk])

        if fp8:
            fp8_bias = persist.tile([PB, 1], F32, name="fp8_bias")
            nc.vector.memset(fp8_bias[:], FP8_EXP_BIAS)
        # masked ones column: va[:, jb*vw + C] = mask01[:, jb]
        va_ones = va_sb[:].rearrange("p (j e) -> p j e", e=vw)[:, :, C:C + 1]
        nc.vector.tensor_copy(va_ones, mb_sb[:].rearrange("p (j e) -> p j e", e=1))

        # ---- projections ----
        with tc.tile_pool(name="proj_psum", bufs=2, space="PSUM") as pp:
            # q^T[d, t] / k^T[d, t]: lhsT = W^T chunk [c, d], rhs = x^T [c, t]
            for w_sb, x_src, x_w, dst, copy_eng in (
                (wq_sb, xq_sb, TQ, qt_sb, nc.vector.tensor_copy),
                (wk_sb, xt_sb, tk, kt_sb, nc.scalar.copy),
            ):
                nblk = x_w // 512
                widths = [512] * nblk + ([x_w - nblk * 512] if x_w % 512 else [])
                off = 0
                for wdt in widths:
                    for dc in range(NCCH):
                        ps = pp.tile([PB, 512], F32, tag="proj", name="proj_ps")
                        for cc in range(NCCH):
                            nc.tensor.matmul(
                                ps[:, 0:wdt],
                                lhsT=w_sb[:, cc * C + dc * PB: cc * C + (dc + 1) * PB],
                                rhs=x_src[:, cc * x_w + off: cc * x_w + off + wdt],
                                start=(cc == 0),
                                stop=(cc == NCCH - 1),
                            )
                        copy_eng(dst[:, dc * x_w + off: dc * x_w + off + wdt],
                                 ps[:, 0:wdt])
                    off += wdt
            # v[t, d]: lhsT = x^T chunk [c, t-block], rhs = W^T chunk [c, d].
            # xt is host-compacted (only valid keys, zero pad), so v pad rows
            # are 0 and the ones column carries the pad mask.
            for jb in range(njb):
                ps = pp.tile([PB, C], F32, tag="projv", name="projv_ps")
                for cc in range(NCCH):
                    nc.tensor.matmul(
                        ps,
                        lhsT=xt_sb[:, cc * tk + jb * PB: cc * tk + (jb + 1) * PB],
                        rhs=wv_sb[:, cc * C:(cc + 1) * C],
                        start=(cc == 0),
                        stop=(cc == NCCH - 1),
                    )
                nc.vector.tensor_copy(va_sb[:, jb * vw: jb * vw + C], ps)

        # ---- attention main loop ----
        scp = ctx.enter_context(tc.tile_pool(name="sc_psum", bufs=3, space="PSUM"))
        op = ctx.enter_context(tc.tile_pool(name="o_psum", bufs=1, space="PSUM"))
        ppool = ctx.enter_context(tc.tile_pool(name="p_pool", bufs=4))
        fin = ctx.enter_context(tc.tile_pool(name="fin", bufs=3))

        if mode == "projonly":
            os_t = fin.tile([PB, C], F32, tag="os", name="os_t")
            nc.vector.tensor_copy(os_t, kt_sb[:, 0:C])
            nc.sync.dma_start(out[0:PB, :], os_t)
            return
        if mode == "noscores":
            p_static = persist.tile([PB, 4 * SBW], BF16, name="p_static")
            nc.vector.memset(p_static[:], 1.0)

        for sb in range(NSB):
            if mode == "noout":
                op_tiles = None
            else:
                op_tiles = [op.tile([PB, VW], F32, tag=f"o{qb}", name=f"opsum{qb}",
                                    bufs=2 if qb == 0 else 1)
                            for qb in range(NQB)]
            p_tiles = {}

            def emit_scores(jb, sb=sb, p_tiles=p_tiles):
                ps = scp.tile([PB, SBW], F32, tag="sc", name="sc_ps")
                for cc in range(NCCH):
                    nc.tensor.matmul(
                        ps,
                        lhsT=kt_sb[:, cc * tk + jb * PB: cc * tk + (jb + 1) * PB],
                        rhs=qt_sb[:, cc * TQ + sb * SBW: cc * TQ + (sb + 1) * SBW],
                        start=(cc == 0),
                        stop=(cc == NCCH - 1),
                    )
                if fp8:
                    # p for a key-block pair lives in one [128, 2*SBW] tile so
                    # the pair forms a DoubleRow stationary [128, 2, 128].
                    if jb % 2 == 0:
                        pt = ppool.tile([PB, 2 * SBW], FP8, tag="p", name="p_t")
                        p_tiles[jb // 2] = pt
                    else:
                        pt = p_tiles[jb // 2]
                    nc.scalar.activation(
                        pt[:, (jb % 2) * SBW:(jb % 2 + 1) * SBW], ps,
                        mybir.ActivationFunctionType.Exp,
                        bias=fp8_bias[:], scale=SCALE)
                else:
                    pt = ppool.tile([PB, SBW], BF16, tag="p", name="p_t")
                    nc.scalar.activation(
                        pt, ps, mybir.ActivationFunctionType.Exp, scale=SCALE)
                    p_tiles[jb] = pt

            def emit_out(jb, op_tiles=op_tiles, p_tiles=p_tiles):
                pt = p_tiles.pop(jb)
                for qb in range(NQB):
                    nc.tensor.matmul(
                        op_tiles[qb],
                        lhsT=pt[:, qb * PB:(qb + 1) * PB],
                        rhs=va_sb[:, jb * VW:(jb + 1) * VW],
                        start=(jb == 0),
                        stop=(jb == njb - 1),
                    )

            def emit_out_fp8(jp, op_tiles=op_tiles, p_tiles=p_tiles):
                # one DoubleRow matmul contracts both key blocks of the pair
                pt = p_tiles.pop(jp)
                pt3 = pt[:].rearrange("p (n c) -> p n c", n=2)
                va3 = va_sb[:, 2 * jp * VWP:(2 * jp + 2) * VWP].rearrange(
                    "p (n c) -> p n c", n=2)
                for qb in range(NQB):
                    nc.tensor.matmul(
                        op_tiles[qb],
                        lhsT=pt3[:, :, qb * PB:(qb + 1) * PB],
                        rhs=va3[:, :, 0:VW],
                        start=(jp == 0),
                        stop=(jp == njb // 2 - 1),
                        perf_mode=mybir.MatmulPerfMode.DoubleRow,
                    )

            if mode == "noout":
                for jb in range(njb):
                    emit_scores(jb)
                    p_tiles.pop(jb)
            elif mode == "noscores":
                for jb in range(njb):
                    for qb in range(NQB):
                        nc.tensor.matmul(
                            op_tiles[qb],
                            lhsT=p_static[:, (jb % 4) * SBW + qb * PB:
                                          (jb % 4) * SBW + (qb + 1) * PB],
                            rhs=va_sb[:, jb * VW:(jb + 1) * VW],
                            start=(jb == 0),
                            stop=(jb == njb - 1),
                        )
            else:
                # software-pipelined: scores/exp for jp+1 are emitted before
                # the out-matmuls of jp so PE never stalls on ACT.
                if fp8:
                    emit_scores(0)
                    emit_scores(1)
                    for jp in range(njb // 2):
                        if 2 * jp + 2 < njb:
                            emit_scores(2 * jp + 2)
                            emit_scores(2 * jp + 3)
                        emit_out_fp8(jp)
                else:
                    emit_scores(0)
                    for jb in range(njb):
                        if jb + 1 < njb:
                            emit_scores(jb + 1)
                        emit_out(jb)
            if mode == "noout":
                os_t = fin.tile([PB, C], F32, tag="os", name="os_t")
                nc.vector.tensor_copy(os_t, kt_sb[:, sb * C:(sb + 1) * C])
                nc.sync.dma_start(out[sb * PB:(sb + 1) * PB, :], os_t)
                continue
            os_t = fin.tile([PB, NQB * C], F32, tag="os", name="os_t")
            for qb in range(NQB):
                rec = fin.tile([PB, 1], F32, tag="rec", name="rec_t")
                nc.vector.reciprocal(rec, op_tiles[qb][:, C:C + 1])
                nc.vector.tensor_scalar_mul(
                    os_t[:, qb * C:(qb + 1) * C], op_tiles[qb][:, 0:C], rec)
            dma_eng = nc.sync if sb % 2 == 0 else nc.scalar
            dma_eng.dma_start(
                out[sb * SBW:(sb + 1) * SBW, :].rearrange("(q p) c -> p q c", p=PB),
                os_t[:].rearrange("p (q c) -> p q c", q=NQB))


def build_nc(reps=1, loop_n=0, mode="full", fp8=False, tk=TK):
    nc = bacc.Bacc("TRN2", target_bir_lowering=False, debug=False)
    xt = nc.dram_tensor("xt", [C, tk], BF16, kind="ExternalInput").ap()
    xq = nc.dram_tensor("xq", [C, TQ], BF16, kind="ExternalInput").ap()
    wq = nc.dram_tensor("wq", [C, C], BF16, kind="ExternalInput").ap()
    wk = nc.dram_tensor("wk", [C, C], BF16, kind="ExternalInput").ap()
    wv = nc.dram_tensor("wv", [C, C], BF16, kind="ExternalInput").ap()
    mb = nc.dram_tensor("mb", [PB, tk // PB], F32, kind="ExternalInput").ap()
    out = nc.dram_tensor("out", [TQ, C], F32, kind="ExternalOutput").ap()
    with tile.TileContext(nc) as tc:
        if loop_n:
            with tc.For_i(0, loop_n, 1, hint_engines=(mybir.EngineType.PE,)):
                _emit(tc, out, xt, xq, wq, wk, wv, mb, tk, mode=mode, fp8=fp8)
        else:
            for _ in range(reps):
                _emit(tc, out, xt, xq, wq, wk, wv, mb, tk, mode=mode, fp8=fp8)
    nc.compile()
    return nc


_CACHE = {}


def _get_nc(tk=TK):
    key = ("nc", tk)
    if key not in _CACHE:
        _CACHE[key] = build_nc(tk=tk)
    return _CACHE[key]


def make_in_maps(x, mask, tk=None):
    bf = ml_dtypes.bfloat16
    x = np.asarray(x, dtype=np.float32)
    m = np.asarray(mask) != 0                                    # [B, T]
    counts = m.sum(axis=1)
    if tk is None:
        tk = TK if counts.max() <= TK else T                     # fallback: no compaction
    xt_all = np.ascontiguousarray(x.transpose(0, 2, 1)).astype(bf)  # [B, C, T]
    maps = []
    xtc_all, mbc_all = [], []
    for b in range(B):
        idx = np.nonzero(m[b])[0]
        nv = len(idx)
        xtc = np.zeros((C, tk), dtype=bf)
        xtc[:, :nv] = xt_all[b][:, idx]
        mbc = np.zeros(tk, dtype=np.float32)
        mbc[:nv] = 1.0
        xtc_all.append(xtc)
        mbc_all.append(np.ascontiguousarray(mbc.reshape(tk // PB, PB).T))
    for core in range(NCORES):
        b, h = divmod(core, HALVES)
        maps.append({
            "xt": xtc_all[b],
            "xq": np.ascontiguousarray(xt_all[b][:, h * TQ:(h + 1) * TQ]),
            "mb": mbc_all[b],
        })
    return maps, tk


def kernel(x, mask, Wk, Wq, Wv):
    bf = ml_dtypes.bfloat16
    wqt = np.ascontiguousarray(np.asarray(Wq, dtype=np.float32).T).astype(bf)
    wkt = np.ascontiguousarray(np.asarray(Wk, dtype=np.float32).T).astype(bf)
    wvt = np.ascontiguousarray(np.asarray(Wv, dtype=np.float32).T).astype(bf)
    in_maps, tk = make_in_maps(x, mask)
    for m in in_maps:
        m.update({"wq": wqt, "wk": wkt, "wv": wvt})
    res = run_bass_kernel_spmd(_get_nc(tk), in_maps, list(range(NCORES)))
    out = np.empty((B, T, C), np.float32)
    for core in range(NCORES):
        b, h = divmod(core, HALVES)
        out[b, h * TQ:(h + 1) * TQ, :] = res.results[core]["out"]
    return out


# revision 9
# speedup vs baseline: 1.5563x; 1.1447x over previous
"""Bass/Tile Trainium2 kernel for nn_Attention (B=4, T=4096, C=256), 8 cores.

Sharding: core = (batch b, query-half h). Each core computes the full K/V
projections for its batch and attention output for its 2048 query rows.

Key compaction: the 0/1 key mask keeps ~50% of keys. The host gathers the
valid key columns of x^T per batch (padded with zeros to TK), so the device
only projects/attends over TK=2176 keys instead of T=4096 — softmax over
the compacted key set is exact (the torch +1.0-on-valid-keys quirk is a
uniform shift that cancels; padding keys have v=0 and a zeroed ones-column
entry so they drop out of both softmax sums). Falls back to a full-T build
if a batch ever has more than TK valid keys.

Layout strategy (all matmuls bf16, fp32 PSUM accumulation):
  - Host pre-transposes x to x^T [C, T]; projections contract C on
    partitions. k^T/q^T come out feature-major, so the score matmul
    produces scoresT [keys j on partitions, queries q on free dim].
  - Softmax needs no max-subtraction (scores are O(1); exp cannot
    overflow fp32) and no partition reductions.
  - V gets a column of ones appended: out[q, 256] accumulates the
    softmax denominator for free. Final: out[:, :256] * (1/out[:, 256]).
  - Main loop is software-pipelined per key block: PE does the two score
    matmuls for block jb+1 and then the four out-matmuls for block jb,
    so ACT's exp (~720 ns/tile) hides behind PE work.
"""

import numpy as np
import ml_dtypes

import concourse.bacc as bacc
import concourse.mybir as mybir
import concourse.tile as tile
from concourse.bass_utils import run_bass_kernel_spmd

B, T, C = 4, 4096, 256
NCORES = 8
HALVES = NCORES // B          # 2 query-halves per batch
TQ = T // HALVES              # 2048 query rows per core
PB = 128                      # partition block
NCCH = C // PB                # 2 contraction chunks of 128
TK = 2176                     # compacted+padded key count (17 blocks of 128)
SBW = 512                     # query superblock width
NSB = TQ // SBW               # 4 superblocks per core
NQB = SBW // PB               # 4 query 128-blocks per superblock
VW = C + 1                    # v tile width incl. ones column
SCALE = float(C) ** -0.5
BF16 = mybir.dt.bfloat16
F32 = mybir.dt.float32
FP8 = mybir.dt.float8e4
VWP = 272                     # fp8 va block pitch (16B-aligned for DoubleRow)
FP8_EXP_BIAS = -6.0           # exp shift so p fits fp8e4m3 range; cancels in softmax


def _emit(tc, out, xt, xq, wq, wk, wv, mb, tk, mode="full", fp8=False):
    nc = tc.nc
    import contextlib
    njb = tk // PB            # key blocks
    nks = tk // 512           # full 512-wide k-proj blocks
    ktail = tk - nks * 512    # k-proj tail width (multiple of 128)

    with contextlib.ExitStack() as ctx:
        persist = ctx.enter_context(tc.tile_pool(name="persist", bufs=1))
        # Persistent SBUF tensors; c-chunks laid side by side on the free dim.
        xt_sb = persist.tile([PB, NCCH * tk], BF16)   # x^T  (compacted keys)
        xq_sb = persist.tile([PB, NCCH * TQ], BF16)   # x^T  (this core's half)
        wq_sb = persist.tile([PB, NCCH * C], BF16)
        wk_sb = persist.tile([PB, NCCH * C], BF16)
        wv_sb = persist.tile([PB, NCCH * C], BF16)
        kt_sb = persist.tile([PB, NCCH * tk], BF16)   # k^T
        qt_sb = persist.tile([PB, NCCH * TQ], BF16)   # q^T
        vdt, vw = (FP8, VWP) if fp8 else (BF16, VW)
        va_sb = persist.tile([PB, njb * vw], vdt)     # masked v + masked ones col
        mb_sb = persist.tile([PB, njb], F32)          # 0/1 mask, [j in block, jb]

        # Few, large, descriptor-friendly DMAs spread across the three
        # DMA-capable queues (sync/scalar HWDGE, gpsimd SWDGE). xq and
        # weights land first so the q projection starts while xt streams.
        w2 = lambda w: w.rearrange("(n p) c -> p n c", p=PB)
        s3 = lambda t, n: t.rearrange("p (n c) -> p n c", n=n)
        nc.scalar.dma_start(s3(wq_sb[:], NCCH), w2(wq))
        nc.scalar.dma_start(s3(wk_sb[:], NCCH), w2(wk))
        nc.gpsimd.dma_start(s3(wv_sb[:], NCCH), w2(wv))
        nc.gpsimd.dma_start(mb_sb[:], mb)
        xq3 = xq.rearrange("(n p) t -> p n t", p=PB)
        # xq in pieces so the q-projection's first blocks start early
        for lo, hi in ((0, 512), (512, 1024), (1024, TQ)):
            dst = xq_sb[:].rearrange("p (n t) -> p n t", n=NCCH)[:, :, lo:hi]
            nc.sync.dma_start(dst, xq3[:, :, lo:hi])
        H = tk // 2
        nc.sync.dma_start(xt_sb[:, 0:H], xt[0:PB, 0:H])
        nc.scalar.dma_start(xt_sb[:, tk:tk + H], xt[PB:2 * PB, 0:H])
        nc.sync.dma_start(xt_sb[:, H:tk], xt[0:PB, H:tk])
        nc.scalar.dma_start(xt_sb[:, tk + H:2 * tk], xt[PB:2 * PB, H:tk])

        if fp8:
            fp8_bias = persist.tile([PB, 1], F32, name="fp8_bias")
            nc.vector.memset(fp8_bias[:], FP8_EXP_BIAS)
        # masked ones column: va[:, jb*vw + C] = mask01[:, jb]
        va_ones = va_sb[:].rearrange("p (j e) -> p j e", e=vw)[:, :, C:C + 1]
        nc.vector.tensor_copy(va_ones, mb_sb[:].rearrange("p (j e) -> p j e", e=1))

        # ---- projections ----
        # The f32 PSUM -> bf16 SBUF copies are the proj-phase bottleneck;
        # round-robin them across DVE and ACT to balance the two engines.
        cp_engs = (nc.vector.tensor_copy, nc.scalar.copy)
        cp_i = [0]

        def cp(dst, src):
            cp_engs[cp_i[0] % 2](dst, src)
            cp_i[0] += 1

        with tc.tile_pool(name="proj_psum", bufs=2, space="PSUM") as pp:
            # q^T[d, t] / k^T[d, t]: lhsT = W^T chunk [c, d], rhs = x^T [c, t]
            for w_sb, x_src, x_w, dst in (
                (wq_sb, xq_sb, TQ, qt_sb),
                (wk_sb, xt_sb, tk, kt_sb),
            ):
                nblk = x_w // 512
                widths = [512] * nblk + ([x_w - nblk * 512] if x_w % 512 else [])
                off = 0
                for wdt in widths:
                    for dc in range(NCCH):
                        ps = pp.tile([PB, 512], F32, tag="proj", name="proj_ps")
                        for cc in range(NCCH):
                            nc.tensor.matmul(
                                ps[:, 0:wdt],
                                lhsT=w_sb[:, cc * C + dc * PB: cc * C + (dc + 1) * PB],
                                rhs=x_src[:, cc * x_w + off: cc * x_w + off + wdt],
                                start=(cc == 0),
                                stop=(cc == NCCH - 1),
                            )
                        cp(dst[:, dc * x_w + off: dc * x_w + off + wdt],
                           ps[:, 0:wdt])
                    off += wdt
            # v[t, d]: lhsT = x^T chunk [c, t-block], rhs = W^T chunk [c, d].
            # xt is host-compacted (only valid keys, zero pad), so v pad rows
            # are 0 and the ones column carries the pad mask. Two key blocks
            # share one PSUM bank so each copy moves 512 columns.
            for jp in range(0, njb, 2):
                pair = min(2, njb - jp)
                ps = pp.tile([PB, 512], F32, tag="projv", name="projv_ps")
                for j in range(pair):
                    for cc in range(NCCH):
                        nc.tensor.matmul(
                            ps[:, j * C:(j + 1) * C],
                            lhsT=xt_sb[:, cc * tk + (jp + j) * PB:
                                       cc * tk + (jp + j + 1) * PB],
                            rhs=wv_sb[:, cc * C:(cc + 1) * C],
                            start=(cc == 0),
                            stop=(cc == NCCH - 1),
                        )
                dstv = va_sb[:, jp * vw:(jp + pair) * vw].rearrange(
                    "p (j e) -> p j e", e=vw)[:, :, 0:C]
                srcv = ps[:, 0:pair * C].rearrange("p (j e) -> p j e", e=C)
                cp(dstv, srcv)

        # ---- attention main loop ----
        scp = ctx.enter_context(tc.tile_pool(name="sc_psum", bufs=3, space="PSUM"))
        op = ctx.enter_context(tc.tile_pool(name="o_psum", bufs=1, space="PSUM"))
        ppool = ctx.enter_context(tc.tile_pool(name="p_pool", bufs=4))
        fin = ctx.enter_context(tc.tile_pool(name="fin", bufs=3))

        if mode == "projonly":
            os_t = fin.tile([PB, C], F32, tag="os", name="os_t")
            nc.vector.tensor_copy(os_t, kt_sb[:, 0:C])
            nc.sync.dma_start(out[0:PB, :], os_t)
            return
        if mode == "noscores":
            p_static = persist.tile([PB, 4 * SBW], BF16, name="p_static")
            nc.vector.memset(p_static[:], 1.0)

        for sb in range(NSB):
            if mode == "noout":
                op_tiles = None
            else:
                op_tiles = [op.tile([PB, VW], F32, tag=f"o{qb}", name=f"opsum{qb}",
                                    bufs=2 if qb == 0 else 1)
                            for qb in range(NQB)]
            p_tiles = {}

            def emit_scores(jb, sb=sb, p_tiles=p_tiles):
                ps = scp.tile([PB, SBW], F32, tag="sc", name="sc_ps")
                for cc in range(NCCH):
                    nc.tensor.matmul(
                        ps,
                        lhsT=kt_sb[:, cc * tk + jb * PB: cc * tk + (jb + 1) * PB],
                        rhs=qt_sb[:, cc * TQ + sb * SBW: cc * TQ + (sb + 1) * SBW],
                        start=(cc == 0),
                        stop=(cc == NCCH - 1),
                    )
                if fp8:
                    # p for a key-block pair lives in one [128, 2*SBW] tile so
                    # the pair forms a DoubleRow stationary [128, 2, 128].
                    if jb % 2 == 0:
                        pt = ppool.tile([PB, 2 * SBW], FP8, tag="p", name="p_t")
                        p_tiles[jb // 2] = pt
                    else:
                        pt = p_tiles[jb // 2]
                    nc.scalar.activation(
                        pt[:, (jb % 2) * SBW:(jb % 2 + 1) * SBW], ps,
                        mybir.ActivationFunctionType.Exp,
                        bias=fp8_bias[:], scale=SCALE)
                else:
                    pt = ppool.tile([PB, SBW], BF16, tag="p", name="p_t")
                    nc.scalar.activation(
                        pt, ps, mybir.ActivationFunctionType.Exp, scale=SCALE)
                    p_tiles[jb] = pt

            def emit_out(jb, op_tiles=op_tiles, p_tiles=p_tiles):
                pt = p_tiles.pop(jb)
                for qb in range(NQB):
                    nc.tensor.matmul(
                        op_tiles[qb],
                        lhsT=pt[:, qb * PB:(qb + 1) * PB],
                        rhs=va_sb[:, jb * VW:(jb + 1) * VW],
                        start=(jb == 0),
                        stop=(jb == njb - 1),
                    )

            def emit_out_fp8(jp, op_tiles=op_tiles, p_tiles=p_tiles):
                # one DoubleRow matmul contracts both key blocks of the pair
                pt = p_tiles.pop(jp)
                pt3 = pt[:].rearrange("p (n c) -> p n c", n=2)
                va3 = va_sb[:, 2 * jp * VWP:(2 * jp + 2) * VWP].rearrange(
                    "p (n c) -> p n c", n=2)
                for qb in range(NQB):
                    nc.tensor.matmul(
                        op_tiles[qb],
                        lhsT=pt3[:, :, qb * PB:(qb + 1) * PB],
                        rhs=va3[:, :, 0:VW],
                        start=(jp == 0),
                        stop=(jp == njb // 2 - 1),
                        perf_mode=mybir.MatmulPerfMode.DoubleRow,
                    )

            if mode == "noout":
                for jb in range(njb):
                    emit_scores(jb)
                    p_tiles.pop(jb)
            elif mode == "noscores":
                for jb in range(njb):
                    for qb in range(NQB):
                        nc.tensor.matmul(
                            op_tiles[qb],
                            lhsT=p_static[:, (jb % 4) * SBW + qb * PB:
                                          (jb % 4) * SBW + (qb + 1) * PB],
                            rhs=va_sb[:, jb * VW:(jb + 1) * VW],
                            start=(jb == 0),
                            stop=(jb == njb - 1),
                        )
            else:
                # software-pipelined: scores/exp for jp+1 are emitted before
                # the out-matmuls of jp so PE never stalls on ACT.
                if fp8:
                    emit_scores(0)
                    emit_scores(1)
                    for jp in range(njb // 2):
                        if 2 * jp + 2 < njb:
                            emit_scores(2 * jp + 2)
                            emit_scores(2 * jp + 3)
                        emit_out_fp8(jp)
                else:
                    # depth-2 score pipeline: exp(jb) has two score-matmul
                    # durations of PE slack to finish before out(jb) needs it.
                    emit_scores(0)
                    emit_scores(1)
                    for jb in range(njb):
                        if jb + 2 < njb:
                            emit_scores(jb + 2)
                        emit_out(jb)
            if mode == "noout":
                os_t = fin.tile([PB, C], F32, tag="os", name="os_t")
                nc.vector.tensor_copy(os_t, kt_sb[:, sb * C:(sb + 1) * C])
                nc.sync.dma_start(out[sb * PB:(sb + 1) * PB, :], os_t)
                continue
            os_t = fin.tile([PB, NQB * C], F32, tag="os", name="os_t")
            for qb in range(NQB):
                rec = fin.tile([PB, 1], F32, tag="rec", name="rec_t")
                nc.vector.reciprocal(rec, op_tiles[qb][:, C:C + 1])
                nc.vector.tensor_scalar_mul(
                    os_t[:, qb * C:(qb + 1) * C], op_tiles[qb][:, 0:C], rec)
            # keep outputs off the sync/scalar queues that carry the next
            # iteration's input DMAs (For_i loop), so heads don't queue
            # behind tails.
            dma_eng = nc.gpsimd if sb % 2 == 0 else nc.scalar
            dma_eng.dma_start(
                out[sb * SBW:(sb + 1) * SBW, :].rearrange("(q p) c -> p q c", p=PB),
                os_t[:].rearrange("p (q c) -> p q c", q=NQB))


def build_nc(reps=1, loop_n=0, mode="full", fp8=False, tk=TK):
    nc = bacc.Bacc("TRN2", target_bir_lowering=False, debug=False)
    xt = nc.dram_tensor("xt", [C, tk], BF16, kind="ExternalInput").ap()
    xq = nc.dram_tensor("xq", [C, TQ], BF16, kind="ExternalInput").ap()
    wq = nc.dram_tensor("wq", [C, C], BF16, kind="ExternalInput").ap()
    wk = nc.dram_tensor("wk", [C, C], BF16, kind="ExternalInput").ap()
    wv = nc.dram_tensor("wv", [C, C], BF16, kind="ExternalInput").ap()
    mb = nc.dram_tensor("mb", [PB, tk // PB], F32, kind="ExternalInput").ap()
    out = nc.dram_tensor("out", [TQ, C], F32, kind="ExternalOutput").ap()
    with tile.TileContext(nc) as tc:
        if loop_n:
            with tc.For_i(0, loop_n, 1, hint_engines=(mybir.EngineType.PE,)):
                _emit(tc, out, xt, xq, wq, wk, wv, mb, tk, mode=mode, fp8=fp8)
        else:
            for _ in range(reps):
                _emit(tc, out, xt, xq, wq, wk, wv, mb, tk, mode=mode, fp8=fp8)
    nc.compile()
    return nc


_CACHE = {}


def _get_nc(tk=TK):
    key = ("nc", tk)
    if key not in _CACHE:
        _CACHE[key] = build_nc(tk=tk)
    return _CACHE[key]


def make_in_maps(x, mask, tk=None):
    bf = ml_dtypes.bfloat16
    x = np.asarray(x, dtype=np.float32)
    m = np.asarray(mask) != 0                                    # [B, T]
    counts = m.sum(axis=1)
    if tk is None:
        tk = TK if counts.max() <= TK else T                     # fallback: no compaction
    xt_all = np.ascontiguousarray(x.transpose(0, 2, 1)).astype(bf)  # [B, C, T]
    maps = []
    xtc_all, mbc_all = [], []
    for b in range(B):
        idx = np.nonzero(m[b])[0]
        nv = len(idx)
        xtc = np.zeros((C, tk), dtype=bf)
        xtc[:, :nv] = xt_all[b][:, idx]
        mbc = np.zeros(tk, dtype=np.float32)
        mbc[:nv] = 1.0
        xtc_all.append(xtc)
        mbc_all.append(np.ascontiguousarray(mbc.reshape(tk // PB, PB).T))
    for core in range(NCORES):
        b, h = divmod(core, HALVES)
        maps.append({
            "xt": xtc_all[b],
            "xq": np.ascontiguousarray(xt_all[b][:, h * TQ:(h + 1) * TQ]),
            "mb": mbc_all[b],
        })
    return maps, tk


def kernel(x, mask, Wk, Wq, Wv):
    bf = ml_dtypes.bfloat16
    wqt = np.ascontiguousarray(np.asarray(Wq, dtype=np.float32).T).astype(bf)
    wkt = np.ascontiguousarray(np.asarray(Wk, dtype=np.float32).T).astype(bf)
    wvt = np.ascontiguousarray(np.asarray(Wv, dtype=np.float32).T).astype(bf)
    in_maps, tk = make_in_maps(x, mask)
    for m in in_maps:
        m.update({"wq": wqt, "wk": wkt, "wv": wvt})
    res = run_bass_kernel_spmd(_get_nc(tk), in_maps, list(range(NCORES)))
    out = np.empty((B, T, C), np.float32)
    for core in range(NCORES):
        b, h = divmod(core, HALVES)
        out[b, h * TQ:(h + 1) * TQ, :] = res.results[core]["out"]
    return out


# revision 10
# speedup vs baseline: 1.6125x; 1.0361x over previous
"""Bass/Tile Trainium2 kernel for nn_Attention (B=4, T=4096, C=256), 8 cores.

Sharding: core = (batch b, query-half h). Each core computes the full K/V
projections for its batch and attention output for its 2048 query rows.

Key compaction: the 0/1 key mask keeps ~50% of keys. The host gathers the
valid key columns of x^T per batch (padded with zeros to TK), so the device
only projects/attends over TK=2176 keys instead of T=4096 — softmax over
the compacted key set is exact (the torch +1.0-on-valid-keys quirk is a
uniform shift that cancels; padding keys have v=0 and a zeroed ones-column
entry so they drop out of both softmax sums). Falls back to a full-T build
if a batch ever has more than TK valid keys.

Layout strategy (all matmuls bf16, fp32 PSUM accumulation):
  - Host pre-transposes x to x^T [C, T]; projections contract C on
    partitions. k^T/q^T come out feature-major, so the score matmul
    produces scoresT [keys j on partitions, queries q on free dim].
  - Softmax needs no max-subtraction (scores are O(1); exp cannot
    overflow fp32) and no partition reductions.
  - V gets a column of ones appended: out[q, 256] accumulates the
    softmax denominator for free. Final: out[:, :256] * (1/out[:, 256]).
  - Main loop is software-pipelined per key block: PE does the two score
    matmuls for block jb+1 and then the four out-matmuls for block jb,
    so ACT's exp (~720 ns/tile) hides behind PE work.
"""

import numpy as np
import ml_dtypes

import concourse.bacc as bacc
import concourse.mybir as mybir
import concourse.tile as tile
from concourse.bass_utils import run_bass_kernel_spmd

B, T, C = 4, 4096, 256
NCORES = 8
HALVES = NCORES // B          # 2 query-halves per batch
TQ = T // HALVES              # 2048 query rows per core
PB = 128                      # partition block
NCCH = C // PB                # 2 contraction chunks of 128
TK = 2176                     # compacted+padded key count (17 blocks of 128)
SBW = 512                     # query superblock width
NSB = TQ // SBW               # 4 superblocks per core
NQB = SBW // PB               # 4 query 128-blocks per superblock
VW = C + 1                    # v tile width incl. ones column
SCALE = float(C) ** -0.5
BF16 = mybir.dt.bfloat16
F32 = mybir.dt.float32
FP8 = mybir.dt.float8e4
VWP = 272                     # fp8 va block pitch (16B-aligned for DoubleRow)
FP8_EXP_BIAS = -6.0           # exp shift so p fits fp8e4m3 range; cancels in softmax


def _emit(tc, out, xt, xq, mt, wv, mb, tk, mode="full", fp8=False):
    nc = tc.nc
    import contextlib
    njb = tk // PB            # key blocks

    with contextlib.ExitStack() as ctx:
        persist = ctx.enter_context(tc.tile_pool(name="persist", bufs=1))
        # Persistent SBUF tensors; c-chunks laid side by side on the free dim.
        xt_sb = persist.tile([PB, NCCH * tk], BF16)   # x^T  (compacted keys)
        xq_sb = persist.tile([PB, NCCH * TQ], BF16)   # x^T  (this core's half)
        mt_sb = persist.tile([PB, NCCH * C], BF16)    # (Wq^T Wk) fused weight
        wv_sb = persist.tile([PB, NCCH * C], BF16)
        xm_sb = persist.tile([PB, NCCH * TQ], BF16)   # M^T x_q  (query-side)
        vdt, vw = (FP8, VWP) if fp8 else (BF16, VW)
        va_sb = persist.tile([PB, njb * vw], vdt)     # masked v + masked ones col
        mb_sb = persist.tile([PB, njb], F32)          # 0/1 mask, [j in block, jb]

        # Few, large, descriptor-friendly DMAs spread across the three
        # DMA-capable queues (sync/scalar HWDGE, gpsimd SWDGE). xq and
        # weights land first so the q projection starts while xt streams.
        w2 = lambda w: w.rearrange("(n p) c -> p n c", p=PB)
        s3 = lambda t, n: t.rearrange("p (n c) -> p n c", n=n)
        nc.scalar.dma_start(s3(mt_sb[:], NCCH), w2(mt))
        nc.gpsimd.dma_start(s3(wv_sb[:], NCCH), w2(wv))
        nc.gpsimd.dma_start(mb_sb[:], mb)
        xq3 = xq.rearrange("(n p) t -> p n t", p=PB)
        # xq in pieces so the q-projection's first blocks start early
        for lo, hi in ((0, 512), (512, 1024), (1024, TQ)):
            dst = xq_sb[:].rearrange("p (n t) -> p n t", n=NCCH)[:, :, lo:hi]
            nc.sync.dma_start(dst, xq3[:, :, lo:hi])
        H = tk // 2
        nc.sync.dma_start(xt_sb[:, 0:H], xt[0:PB, 0:H])
        nc.scalar.dma_start(xt_sb[:, tk:tk + H], xt[PB:2 * PB, 0:H])
        nc.sync.dma_start(xt_sb[:, H:tk], xt[0:PB, H:tk])
        nc.scalar.dma_start(xt_sb[:, tk + H:2 * tk], xt[PB:2 * PB, H:tk])

        if fp8:
            fp8_bias = persist.tile([PB, 1], F32, name="fp8_bias")
            nc.vector.memset(fp8_bias[:], FP8_EXP_BIAS)
        # masked ones column: va[:, jb*vw + C] = mask01[:, jb]
        va_ones = va_sb[:].rearrange("p (j e) -> p j e", e=vw)[:, :, C:C + 1]
        nc.vector.tensor_copy(va_ones, mb_sb[:].rearrange("p (j e) -> p j e", e=1))

        # ---- projections ----
        # The f32 PSUM -> bf16 SBUF copies are the proj-phase bottleneck;
        # round-robin them across DVE and ACT to balance the two engines.
        cp_engs = (nc.vector.tensor_copy, nc.scalar.copy)
        cp_i = [0]

        def cp(dst, src):
            cp_engs[cp_i[0] % 2](dst, src)
            cp_i[0] += 1

        with tc.tile_pool(name="proj_psum", bufs=2, space="PSUM") as pp:
            # xm[c, q] = sum_c' Mt[c', c] xq[c', q]: one fused projection
            # replaces both the q- and k-projections (M = Wq^T Wk from host);
            # the score matmul's stationary side reads raw xt from SBUF.
            for w_sb, x_src, x_w, dst in (
                (mt_sb, xq_sb, TQ, xm_sb),
            ):
                nblk = x_w // 512
                widths = [512] * nblk + ([x_w - nblk * 512] if x_w % 512 else [])
                off = 0
                for wdt in widths:
                    for dc in range(NCCH):
                        ps = pp.tile([PB, 512], F32, tag="proj", name="proj_ps")
                        for cc in range(NCCH):
                            nc.tensor.matmul(
                                ps[:, 0:wdt],
                                lhsT=w_sb[:, cc * C + dc * PB: cc * C + (dc + 1) * PB],
                                rhs=x_src[:, cc * x_w + off: cc * x_w + off + wdt],
                                start=(cc == 0),
                                stop=(cc == NCCH - 1),
                            )
                        cp(dst[:, dc * x_w + off: dc * x_w + off + wdt],
                           ps[:, 0:wdt])
                    off += wdt
            # v[t, d]: lhsT = x^T chunk [c, t-block], rhs = W^T chunk [c, d].
            # xt is host-compacted (only valid keys, zero pad), so v pad rows
            # are 0 and the ones column carries the pad mask. Two key blocks
            # share one PSUM bank so each copy moves 512 columns.
            for jp in range(0, njb, 2):
                pair = min(2, njb - jp)
                ps = pp.tile([PB, 512], F32, tag="projv", name="projv_ps")
                for j in range(pair):
                    for cc in range(NCCH):
                        nc.tensor.matmul(
                            ps[:, j * C:(j + 1) * C],
                            lhsT=xt_sb[:, cc * tk + (jp + j) * PB:
                                       cc * tk + (jp + j + 1) * PB],
                            rhs=wv_sb[:, cc * C:(cc + 1) * C],
                            start=(cc == 0),
                            stop=(cc == NCCH - 1),
                        )
                dstv = va_sb[:, jp * vw:(jp + pair) * vw].rearrange(
                    "p (j e) -> p j e", e=vw)[:, :, 0:C]
                srcv = ps[:, 0:pair * C].rearrange("p (j e) -> p j e", e=C)
                cp(dstv, srcv)

        # ---- attention main loop ----
        scp = ctx.enter_context(tc.tile_pool(name="sc_psum", bufs=3, space="PSUM"))
        op = ctx.enter_context(tc.tile_pool(name="o_psum", bufs=1, space="PSUM"))
        ppool = ctx.enter_context(tc.tile_pool(name="p_pool", bufs=4))
        fin = ctx.enter_context(tc.tile_pool(name="fin", bufs=3))

        if mode == "projonly":
            os_t = fin.tile([PB, C], F32, tag="os", name="os_t")
            nc.vector.tensor_copy(os_t, xm_sb[:, 0:C])
            nc.sync.dma_start(out[0:PB, :], os_t)
            return
        if mode == "noscores":
            p_static = persist.tile([PB, 4 * SBW], BF16, name="p_static")
            nc.vector.memset(p_static[:], 1.0)

        for sb in range(NSB):
            if mode == "noout":
                op_tiles = None
            else:
                op_tiles = [op.tile([PB, VW], F32, tag=f"o{qb}", name=f"opsum{qb}",
                                    bufs=2 if qb == 0 else 1)
                            for qb in range(NQB)]
            p_tiles = {}

            def emit_scores(jb, sb=sb, p_tiles=p_tiles):
                ps = scp.tile([PB, SBW], F32, tag="sc", name="sc_ps")
                for cc in range(NCCH):
                    nc.tensor.matmul(
                        ps,
                        lhsT=xt_sb[:, cc * tk + jb * PB: cc * tk + (jb + 1) * PB],
                        rhs=xm_sb[:, cc * TQ + sb * SBW: cc * TQ + (sb + 1) * SBW],
                        start=(cc == 0),
                        stop=(cc == NCCH - 1),
                    )
                if fp8:
                    # p for a key-block pair lives in one [128, 2*SBW] tile so
                    # the pair forms a DoubleRow stationary [128, 2, 128].
                    if jb % 2 == 0:
                        pt = ppool.tile([PB, 2 * SBW], FP8, tag="p", name="p_t")
                        p_tiles[jb // 2] = pt
                    else:
                        pt = p_tiles[jb // 2]
                    nc.scalar.activation(
                        pt[:, (jb % 2) * SBW:(jb % 2 + 1) * SBW], ps,
                        mybir.ActivationFunctionType.Exp,
                        bias=fp8_bias[:], scale=SCALE)
                else:
                    pt = ppool.tile([PB, SBW], BF16, tag="p", name="p_t")
                    nc.scalar.activation(
                        pt, ps, mybir.ActivationFunctionType.Exp, scale=SCALE)
                    p_tiles[jb] = pt

            def emit_out(jb, op_tiles=op_tiles, p_tiles=p_tiles):
                pt = p_tiles.pop(jb)
                for qb in range(NQB):
                    nc.tensor.matmul(
                        op_tiles[qb],
                        lhsT=pt[:, qb * PB:(qb + 1) * PB],
                        rhs=va_sb[:, jb * VW:(jb + 1) * VW],
                        start=(jb == 0),
                        stop=(jb == njb - 1),
                    )

            def emit_out_fp8(jp, op_tiles=op_tiles, p_tiles=p_tiles):
                # one DoubleRow matmul contracts both key blocks of the pair
                pt = p_tiles.pop(jp)
                pt3 = pt[:].rearrange("p (n c) -> p n c", n=2)
                va3 = va_sb[:, 2 * jp * VWP:(2 * jp + 2) * VWP].rearrange(
                    "p (n c) -> p n c", n=2)
                for qb in range(NQB):
                    nc.tensor.matmul(
                        op_tiles[qb],
                        lhsT=pt3[:, :, qb * PB:(qb + 1) * PB],
                        rhs=va3[:, :, 0:VW],
                        start=(jp == 0),
                        stop=(jp == njb // 2 - 1),
                        perf_mode=mybir.MatmulPerfMode.DoubleRow,
                    )

            if mode == "noout":
                for jb in range(njb):
                    emit_scores(jb)
                    p_tiles.pop(jb)
            elif mode == "noscores":
                for jb in range(njb):
                    for qb in range(NQB):
                        nc.tensor.matmul(
                            op_tiles[qb],
                            lhsT=p_static[:, (jb % 4) * SBW + qb * PB:
                                          (jb % 4) * SBW + (qb + 1) * PB],
                            rhs=va_sb[:, jb * VW:(jb + 1) * VW],
                            start=(jb == 0),
                            stop=(jb == njb - 1),
                        )
            else:
                # software-pipelined: scores/exp for jp+1 are emitted before
                # the out-matmuls of jp so PE never stalls on ACT.
                if fp8:
                    emit_scores(0)
                    emit_scores(1)
                    for jp in range(njb // 2):
                        if 2 * jp + 2 < njb:
                            emit_scores(2 * jp + 2)
                            emit_scores(2 * jp + 3)
                        emit_out_fp8(jp)
                else:
                    # depth-2 score pipeline: exp(jb) has two score-matmul
                    # durations of PE slack to finish before out(jb) needs it.
                    emit_scores(0)
                    emit_scores(1)
                    for jb in range(njb):
                        if jb + 2 < njb:
                            emit_scores(jb + 2)
                        emit_out(jb)
            if mode == "noout":
                os_t = fin.tile([PB, C], F32, tag="os", name="os_t")
                nc.vector.tensor_copy(os_t, xm_sb[:, sb * C:(sb + 1) * C])
                nc.sync.dma_start(out[sb * PB:(sb + 1) * PB, :], os_t)
                continue
            os_t = fin.tile([PB, NQB * C], F32, tag="os", name="os_t")
            for qb in range(NQB):
                rec = fin.tile([PB, 1], F32, tag="rec", name="rec_t")
                nc.vector.reciprocal(rec, op_tiles[qb][:, C:C + 1])
                nc.vector.tensor_scalar_mul(
                    os_t[:, qb * C:(qb + 1) * C], op_tiles[qb][:, 0:C], rec)
            # keep outputs off the sync/scalar queues that carry the next
            # iteration's input DMAs (For_i loop), so heads don't queue
            # behind tails.
            dma_eng = nc.gpsimd if sb % 2 == 0 else nc.scalar
            dma_eng.dma_start(
                out[sb * SBW:(sb + 1) * SBW, :].rearrange("(q p) c -> p q c", p=PB),
                os_t[:].rearrange("p (q c) -> p q c", q=NQB))


def build_nc(reps=1, loop_n=0, mode="full", fp8=False, tk=TK):
    nc = bacc.Bacc("TRN2", target_bir_lowering=False, debug=False)
    xt = nc.dram_tensor("xt", [C, tk], BF16, kind="ExternalInput").ap()
    xq = nc.dram_tensor("xq", [C, TQ], BF16, kind="ExternalInput").ap()
    mt = nc.dram_tensor("mt", [C, C], BF16, kind="ExternalInput").ap()
    wv = nc.dram_tensor("wv", [C, C], BF16, kind="ExternalInput").ap()
    mb = nc.dram_tensor("mb", [PB, tk // PB], F32, kind="ExternalInput").ap()
    out = nc.dram_tensor("out", [TQ, C], F32, kind="ExternalOutput").ap()
    with tile.TileContext(nc) as tc:
        if loop_n:
            with tc.For_i(0, loop_n, 1, hint_engines=(mybir.EngineType.PE,)):
                _emit(tc, out, xt, xq, mt, wv, mb, tk, mode=mode, fp8=fp8)
        else:
            for _ in range(reps):
                _emit(tc, out, xt, xq, mt, wv, mb, tk, mode=mode, fp8=fp8)
    nc.compile()
    return nc


_CACHE = {}


def _get_nc(tk=TK):
    key = ("nc", tk)
    if key not in _CACHE:
        _CACHE[key] = build_nc(tk=tk)
    return _CACHE[key]


def make_in_maps(x, mask, tk=None):
    bf = ml_dtypes.bfloat16
    x = np.asarray(x, dtype=np.float32)
    m = np.asarray(mask) != 0                                    # [B, T]
    counts = m.sum(axis=1)
    if tk is None:
        tk = TK if counts.max() <= TK else T                     # fallback: no compaction
    xt_all = np.ascontiguousarray(x.transpose(0, 2, 1)).astype(bf)  # [B, C, T]
    maps = []
    xtc_all, mbc_all = [], []
    for b in range(B):
        idx = np.nonzero(m[b])[0]
        nv = len(idx)
        xtc = np.zeros((C, tk), dtype=bf)
        xtc[:, :nv] = xt_all[b][:, idx]
        mbc = np.zeros(tk, dtype=np.float32)
        mbc[:nv] = 1.0
        xtc_all.append(xtc)
        mbc_all.append(np.ascontiguousarray(mbc.reshape(tk // PB, PB).T))
    for core in range(NCORES):
        b, h = divmod(core, HALVES)
        maps.append({
            "xt": xtc_all[b],
            "xq": np.ascontiguousarray(xt_all[b][:, h * TQ:(h + 1) * TQ]),
            "mb": mbc_all[b],
        })
    return maps, tk


def make_wt_maps(Wk, Wq, Wv):
    bf = ml_dtypes.bfloat16
    wq32 = np.asarray(Wq, dtype=np.float32)
    wk32 = np.asarray(Wk, dtype=np.float32)
    # scoresT[k, q] = sum_c xt[c,k] xm[c,q], xm = Mt^T xq, Mt[c',c] = (Wq^T Wk)[c',c]
    mt = np.ascontiguousarray(wq32.T @ wk32).astype(bf)
    wvt = np.ascontiguousarray(np.asarray(Wv, dtype=np.float32).T).astype(bf)
    return {"mt": mt, "wv": wvt}


def kernel(x, mask, Wk, Wq, Wv):
    in_maps, tk = make_in_maps(x, mask)
    wts = make_wt_maps(Wk, Wq, Wv)
    for m in in_maps:
        m.update(wts)
    res = run_bass_kernel_spmd(_get_nc(tk), in_maps, list(range(NCORES)))
    out = np.empty((B, T, C), np.float32)
    for core in range(NCORES):
        b, h = divmod(core, HALVES)
        out[b, h * TQ:(h + 1) * TQ, :] = res.results[core]["out"]
    return out
